# revision 59
# baseline (speedup 1.0000x reference)
"""Sparse attention (template/search) Trainium2 kernel.

Model (per batch b):
  qkv = x @ qkv_w.T                  -> split to q, k, v heads (12 heads, hd=64)
  template tokens   [0, 256)  attend to template keys only
  search   tokens [256, 1280) attend to all 1280 keys
  out = softmax(q k^T / 8) v   per head, concat heads, @ proj_w.T + proj_b

Sharding: data-parallel over batch, one batch per NeuronCore (8 cores).
No collectives needed.

Layout: all transposes + fp32->bf16 casts are done on the HOST, and the
DRAM images are laid out to match the SBUF destinations byte-for-byte, so
every input DMA is a flat [128, range] copy with multi-KB contiguous runs.
Each region is split across BOTH HWDGE rings (sync + scalar share the
~358 GB/s HBM budget) and ordered by first-use deadline, so attention
starts ~6us into the kernel.  The output is written bf16 (upcast to fp32
on the host), halving output DMA traffic.

Per-core structure:
  - q,k computed feature-major into a 2-slot rotating buffer (slot =
    pair%2): qk[P, slot, {q,k}, NTOK] (q pre-scaled by 1/8).
  - v computed token-major, augmented per head as [1 | 63 zeros | v]:
    row 0 of the AV output is the softmax denominator.
  - scores computed TRANSPOSED: S.T[tk, tq] = K_h @ Q_h.T.  The two heads
    of a pair sit on PE row groups 0-63 / 64-127, so their score matmuls
    run CONCURRENTLY (hw-verified: a packed pair costs ~232ns vs 217ns for
    one matmul), filling the two halves of one PSUM tile -> ONE exp
    instruction per (pair, chunk, tk) covers both heads.
  - search is chunk-outer over query chunks so each head's AV accumulator
    is one PSUM bank; the tk loop is software-pipelined: scores(tk+1) and
    a filler run on the PE while exp(tk) runs on ACT, then AV(tk).
  - fillers (qkv / v / proj matmuls) are spread evenly over the tk slots
    so the PE never drains at chunk ends.
  - pair 5 splits its search into (512, 256, 256) query chunks and the
    tail tiles' proj is pre-accumulated (ct 0..4) as fillers, so after the
    very last normalize only 4 tiny ct=5 matmuls + combines + 0.8 MB of
    output DMA remain.
  - normalize fully off the ACT queue: DVE copy PSUM->SBUF, gpsimd
    partition_broadcast of row 0, DVE approx reciprocal, DVE multiply.
  - PE HAM warmup: zero matmuls at t=0 (no DMA dependency) open the clock
    gate while the input DMA streams.
"""

import numpy as np
import ml_dtypes

import concourse.bacc as bacc
import concourse.mybir as mybir
import concourse.tile as tile

P = 128
NTOK = 1280
C = 768
H = 12
HD = 64
NT = 256          # template tokens  [0, NT)
TT = NTOK // P    # 10 token tiles
CT = C // P       # 6 channel tiles
SCALE = HD ** -0.5

F32 = mybir.dt.float32
BF16 = mybir.dt.bfloat16
EXP = mybir.ActivationFunctionType.Exp
MULT = mybir.AluOpType.mult
ADD = mybir.AluOpType.add

# q/k weight block order in the DRAM/SBUF image: pair-0 q and k first
# (the DMA priority prefix), then q1..q5, k1..k5.  Entry i = original
# column-block (of qkv_w.T's 12 leading 128-col blocks) stored at slot i.
WQK_ORDER = [0, 6, 1, 2, 3, 4, 5, 7, 8, 9, 10, 11]
WQK_SLOT = {orig: slot for slot, orig in enumerate(WQK_ORDER)}


def build_nc():
    from contextlib import ExitStack

    nc = bacc.Bacc("TRN2", target_bir_lowering=False, debug=False, num_devices=8)
    # host-prepared flat images (see _prep_in_maps)
    xT_ext = nc.dram_tensor("xT", [P, 2 * CT * C], BF16, kind="ExternalInput")
    wT_ext = nc.dram_tensor("wT", [P, 18 * CT * P], BF16, kind="ExternalInput")
    pwT_ext = nc.dram_tensor("pwT", [P, CT * C], BF16, kind="ExternalInput")
    pb_ext = nc.dram_tensor("pb", [1, C], F32, kind="ExternalInput")
    out_ext = nc.dram_tensor("out", [NTOK, C], BF16, kind="ExternalOutput")

    with tile.TileContext(nc) as tc, ExitStack() as ctx:
        const = ctx.enter_context(tc.tile_pool(name="const", bufs=1))
        big = ctx.enter_context(tc.tile_pool(name="big", bufs=1))

        zb = const.tile([P, P], BF16)
        nc.gpsimd.memset(zb[:], 0.0)
        bias_bc = const.tile([P, C], F32)
        bias_row = const.tile([1, C], F32)

        # x, feature-major, split [tokens 0:512 | tokens 512:1280(+pad)]
        xt = big.tile([P, 2, CT, C], BF16)
        wqk = big.tile([P, 12, CT, P], BF16)   # q/k weights, WQK_ORDER blocks
        wv0 = big.tile([P, CT, 512], BF16)     # v weights cols 0:512 (h 0-7)
        wv1 = big.tile([P, CT, 256], BF16)     # v weights cols 512:768
        pwT = big.tile([P, CT, C], BF16)       # proj_w.T

        # ---- input DMA: flat 2D copies matching the host DRAM image,
        # priority-ordered and split across the two HWDGE queues ----
        xtf = xt[:, :, :, :].rearrange("p a b c -> p (a b c)")
        wqkf = wqk[:, :, :, :].rearrange("p a b c -> p (a b c)")
        wv0f = wv0[:, :, :].rearrange("p a b -> p (a b)")
        wv1f = wv1[:, :, :].rearrange("p a b -> p (a b)")
        pwf = pwT[:, :, :].rearrange("p a b -> p (a b)")
        QK0 = 2 * CT * P          # pair-0 q/k prefix elems
        XH = CT * C               # one x half
        XQ = XH // 2              # one x quarter (3 channel tiles)
        WR = (QK0 + 12 * CT * P) // 2   # midpoint of the q/k remainder
        WV = 12 * CT * P
        # both HWDGE rings share the ~358 GB/s HBM budget; split every
        # ramp-critical region across the two rings so each lands in half
        # the time, ordered by first-use deadline: pair-0 q/k + x half 0
        # (first qkv matmuls, t~6us), x half 1 (template-gap q chunk,
        # t~8us), v weights half 0 (template AV, t~10us), the rest
        nc.sync.dma_start(wqkf[:, 0:QK0], wT_ext.ap()[:, 0:QK0])
        nc.scalar.dma_start(xtf[:, 0:XQ], xT_ext.ap()[:, 0:XQ])
        nc.sync.dma_start(xtf[:, XQ:XH], xT_ext.ap()[:, XQ:XH])
        nc.scalar.dma_start(xtf[:, XH:XH + XQ], xT_ext.ap()[:, XH:XH + XQ])
        nc.sync.dma_start(xtf[:, XH + XQ:], xT_ext.ap()[:, XH + XQ:])
        WVH = WV + CT * 256
        nc.scalar.dma_start(wv0f[:, 0:CT * 256], wT_ext.ap()[:, WV:WVH])
        nc.sync.dma_start(wv0f[:, CT * 256:], wT_ext.ap()[:, WVH:WV + CT * 512])
        nc.scalar.dma_start(wqkf[:, QK0:WR], wT_ext.ap()[:, QK0:WR])
        nc.sync.dma_start(wqkf[:, WR:], wT_ext.ap()[:, WR:12 * CT * P])
        nc.scalar.dma_start(wv1f[:], wT_ext.ap()[:, WV + CT * 512:])
        nc.sync.dma_start(pwf[:], pwT_ext.ap())
        nc.scalar.dma_start(bias_row[:], pb_ext.ap())
        nc.gpsimd.partition_broadcast(bias_bc[:], bias_row[0:1, :])

        def xchunk(ct, c0, cw):
            """x.T [ct*128:(ct+1)*128, c0:c0+cw] from the split layout
            (chunks never straddle the 512 boundary)."""
            if c0 + cw <= 512:
                return xt[:, 0, ct, c0:c0 + cw]
            return xt[:, 1, ct, c0 - 512:c0 - 512 + cw]

        big2 = ctx.enter_context(tc.tile_pool(name="big2", bufs=1))
        # q (scaled) and k, feature-major, 2-slot rotation keyed by pair%2
        qk = big2.tile([P, 2, 2, NTOK], BF16)
        v_sb = big2.tile([P, TT, H, P], BF16)  # [1 | 63 zeros | v] per head
        ot_all = big2.tile([P, CT, NTOK], BF16)     # attention out, feature-major
        out_sb = big2.tile([P, TT, C], BF16)
        # proj partial sums (ct 0..4 terms + bias) for the tail tiles 6..9
        proj_part = big2.tile([P, 4, C], F32)

        # v_aug layout per head: col 0 = ones (softmax denominator row),
        # cols 1:64 = zeros (padding so O lands at partitions 64:128)
        nc.gpsimd.memset(v_sb[:, :, :, 0:64], 0.0)
        nc.gpsimd.memset(v_sb[:, :, :, 0:1], 1.0)

        # ---- HAM warmup: zero matmuls with no DMA dependency keep the PE
        # busy from t=0 so its clock gate opens (1.2 -> 2.4 GHz) before the
        # first real matmul.  Result (zeros) lands in v_sb zero padding to
        # keep the chain live.
        with tc.tile_pool(name="warm", bufs=2, space="PSUM") as ps_warm:
            wps = [ps_warm.tile([P, P], F32, name=f"wps{j}") for j in range(2)]
            for i in range(56):
                nc.tensor.matmul(wps[i % 2][:], zb[:], zb[:],
                                 start=True, stop=True)
            nc.vector.tensor_copy(v_sb[:, 0, 0, 1:64], wps[1][:, 0:63])

        ps_fill = ctx.enter_context(tc.tile_pool(name="ps_fill", bufs=2, space="PSUM"))

        # ---- qkv projection (emitted interleaved with attention below) ----
        def emit_qk_chunk(hp, which, c0, cw):
            """qk[slot, which] = (q|k) row block of head pair hp,
            feature-major, for token chunk [c0, c0+cw)."""
            slot = WQK_SLOT[hp + 6 * which]
            ps = ps_fill.tile([P, 512], F32, tag="fill", name=f"qkp{hp}_{which}_{c0}")
            for ct in range(CT):
                nc.tensor.matmul(
                    ps[:, :cw],
                    wqk[:, slot, ct, :],
                    xchunk(ct, c0, cw),
                    start=(ct == 0), stop=(ct == CT - 1),
                )
            if which == 0:  # q: fold in softmax scale
                nc.vector.tensor_scalar_mul(
                    qk[:, hp % 2, 0, c0:c0 + cw], ps[:, :cw], SCALE
                )
            else:
                nc.vector.tensor_copy(qk[:, hp % 2, 1, c0:c0 + cw], ps[:, :cw])

        def qk_pair_chunks(p):
            return [(p, w, c0, cw)
                    for c0, cw in ((0, 512), (512, 512), (1024, 256))
                    for w in (0, 1)]

        # v token-major: v[tok, f] = x @ qkv_w.T cols [1536, 2304)
        def emit_v_chunk(tt, half):
            cw, h0, nh = ((512, 0, 8), (256, 8, 4))[half]
            wv = (wv0, wv1)[half]
            ps = ps_fill.tile([P, 512], F32, tag="fill", name=f"vp{tt}_{half}")
            for ct in range(CT):
                nc.tensor.matmul(
                    ps[:, :cw],
                    xchunk(ct, tt * P, P),
                    wv[:, ct, :],
                    start=(ct == 0), stop=(ct == CT - 1),
                )
            nc.vector.tensor_copy(
                v_sb[:, tt, h0:h0 + nh, 64:128],
                ps[:, :cw].rearrange("p (h e) -> p h e", e=HD),
            )

        # ---- output projection ----
        def emit_proj_chunk(tt, half):
            c0, cw = ((0, 512), (512, 256))[half]
            ps = ps_fill.tile([P, 512], F32, tag="fill", name=f"prj{tt}_{c0}")
            for ct in range(CT):
                nc.tensor.matmul(
                    ps[:, :cw],
                    ot_all[:, ct, tt * P:(tt + 1) * P],
                    pwT[:, ct, c0:c0 + cw],
                    start=(ct == 0), stop=(ct == CT - 1),
                )
            nc.vector.tensor_tensor(
                out_sb[:, tt, c0:c0 + cw], ps[:, :cw],
                bias_bc[:, c0:c0 + cw], ADD,
            )
            nc.sync.dma_start(out_ext.ap()[tt * P:(tt + 1) * P, c0:c0 + cw],
                              out_sb[:, tt, c0:c0 + cw])

        # proj split for the tail tiles (6..9): ct 0..4 terms accumulate as
        # pair-5 fillers (pairs 0-4 are fully normalized by then); the ct=5
        # term + combine runs after that tile's chunk normalize.
        def emit_proj_partial(tt, half):
            c0, cw = ((0, 512), (512, 256))[half]
            ps = ps_fill.tile([P, 512], F32, tag="fill", name=f"prp{tt}_{c0}")
            for ct in range(CT - 1):
                nc.tensor.matmul(
                    ps[:, :cw],
                    ot_all[:, ct, tt * P:(tt + 1) * P],
                    pwT[:, ct, c0:c0 + cw],
                    start=(ct == 0), stop=(ct == CT - 2),
                )
            nc.vector.tensor_tensor(
                proj_part[:, tt - 6, c0:c0 + cw], ps[:, :cw],
                bias_bc[:, c0:c0 + cw], ADD,
            )

        def emit_proj_final(tt, half):
            c0, cw = ((0, 512), (512, 256))[half]
            ps = ps_fill.tile([P, 512], F32, tag="fill", name=f"prf{tt}_{c0}")
            nc.tensor.matmul(
                ps[:, :cw],
                ot_all[:, CT - 1, tt * P:(tt + 1) * P],
                pwT[:, CT - 1, c0:c0 + cw],
                start=True, stop=True,
            )
            nc.vector.tensor_tensor(
                out_sb[:, tt, c0:c0 + cw], ps[:, :cw],
                proj_part[:, tt - 6, c0:c0 + cw], ADD,
            )
            # tail tiles alternate between the two HWDGE rings to double
            # the end-of-kernel output DMA bandwidth
            eng = nc.scalar if tt % 2 else nc.sync
            eng.dma_start(out_ext.ap()[tt * P:(tt + 1) * P, c0:c0 + cw],
                          out_sb[:, tt, c0:c0 + cw])

        def emit_filler(kind, arg):
            if kind == "qk":
                emit_qk_chunk(*arg)
            elif kind == "v":
                emit_v_chunk(*arg)
            elif kind == "projp":
                emit_proj_partial(*arg)
            elif kind == "projf":
                emit_proj_final(*arg)
            else:
                emit_proj_chunk(*arg)

        # pair-0 q/k c0 chunks + template v tiles up front (these gate the
        # first template block and only need the priority DMAs); the rest
        # of pair-0 q/k streams inside pair 0 itself
        emit_qk_chunk(0, 0, 0, 512)
        emit_qk_chunk(0, 1, 0, 512)
        for tt in (0, 1):
            emit_v_chunk(tt, 0)

        # ---- attention ----
        ps_sc = ctx.enter_context(tc.tile_pool(name="ps_sc", bufs=2, space="PSUM"))
        ps_ot = ctx.enter_context(tc.tile_pool(name="ps_ot", bufs=2, space="PSUM"))
        pts = ctx.enter_context(tc.tile_pool(name="pts", bufs=6))
        dn = ctx.enter_context(tc.tile_pool(name="dn", bufs=2))
        rbp = ctx.enter_context(tc.tile_pool(name="rbp", bufs=2))

        def qh(h, c0, cw):
            b = (h % 2) * 64
            return qk[b:b + 64, (h // 2) % 2, 0, c0:c0 + cw]

        def kh(h, tk):
            b = (h % 2) * 64
            return qk[b:b + 64, (h // 2) % 2, 1, tk * P:(tk + 1) * P]

        def normalize_pair(hs, ot_pss, c0, cw):
            """Per head: ot_ps [128, cw] psum (row 0 = denominators, rows
            64:128 = O.T for tq cols [c0, c0+cw)).  Normalize and write to
            ot_all, fully off the ACT queue.  Both heads' PSUM->SBUF copies
            are issued first so the second head's chain is not blocked
            behind the first head's broadcast wait on the DVE FIFO."""
            dens, rbs = [], []
            for h, ot_ps in zip(hs, ot_pss):
                den = dn.tile([P, 512], F32, tag="dn")
                nc.vector.tensor_copy(den[:, :cw], ot_ps[:, :cw])
                dens.append(den)
            for den in dens:
                rb = rbp.tile([P, 512], F32, tag="rb")
                nc.gpsimd.partition_broadcast(rb[:, :cw], den[0:1, :cw])
                rbs.append(rb)
            for h, den, rb in zip(hs, dens, rbs):
                b = (h % 2) * 64
                nc.vector.reciprocal_approx_fast(rb[:, :cw], rb[:, :cw])
                nc.vector.tensor_tensor(
                    ot_all[b:b + 64, h // 2, c0:c0 + cw],
                    den[64:128, :cw], rb[64:128, :cw], MULT,
                )

        for hp in range(6):
            h0, h1 = 2 * hp, 2 * hp + 1
            # query chunks of this pair's search and their filler lists
            if hp < 5:
                chunks = [(NT, 512), (NT + 512, 512)]
            else:
                chunks = [(NT, 512), (NT + 512, 256), (NT + 768, 256)]
            if hp == 0:
                # pair-0's own remaining q/k interleaves with its search:
                # k(512) needed at tk4, k(1024) at tk8, q(1024) before cj1
                pend = [[("qk", (0, 1, 512, 512)),
                         ("v", (2, 0)), ("v", (3, 0)), ("v", (4, 0)),
                         ("qk", (0, 1, 1024, 256))]
                        + [("v", (tt, 0)) for tt in range(5, TT)]
                        + [("qk", (0, 0, 1024, 256))],
                        [("qk", a) for a in qk_pair_chunks(1)]]
            elif hp < 4:
                nxt = [("qk", a) for a in qk_pair_chunks(hp + 1)]
                vh = {1: [0, 1, 2, 3], 2: [4, 5, 6], 3: [7, 8, 9]}[hp]
                va = [("v", (tt, 1)) for tt in vh]
                pend = [nxt[:3] + va[:2], nxt[3:] + va[2:]]
            elif hp == 4:
                nxt = [("qk", a) for a in qk_pair_chunks(5)]
                pend = [nxt[:3], nxt[3:]]
            else:
                # chunk 0 (q 256:768), chunk A (768:1024 -> tiles 6,7),
                # chunk B (1024:1280 -> tiles 8,9).  proj gates: tiles 0,1
                # by the template normalize just below; tiles 2-5 by chunk-0
                # normalize; prp by pairs 0-4 (already done); prf(6,7) by
                # chunk-A normalize.
                pend = [[("projp", (6, 0)), ("projp", (6, 1)),
                         ("proj", (0, 0)), ("proj", (0, 1)),
                         ("proj", (1, 0)), ("proj", (1, 1)),
                         ("projp", (7, 0)), ("projp", (7, 1)),
                         ("projp", (8, 0)), ("projp", (8, 1))],
                        [("projp", (9, 0)), ("projp", (9, 1)),
                         ("proj", (2, 0)), ("proj", (2, 1)),
                         ("proj", (3, 0)), ("proj", (3, 1)),
                         ("proj", (4, 0)), ("proj", (4, 1))],
                        [("proj", (5, 0)), ("proj", (5, 1)),
                         ("projf", (6, 0)), ("projf", (6, 1)),
                         ("projf", (7, 0)), ("projf", (7, 1))]]

            # template block, both heads fused: queries [0,256) x keys [0,256)
            st_t = ps_sc.tile([P, 1024], F32, tag="sc", name=f"tst{hp}")
            for tj in range(2):
                for hi, h in enumerate((h0, h1)):
                    nc.tensor.matmul(
                        st_t[:, hi * 512 + tj * NT: hi * 512 + (tj + 1) * NT],
                        kh(h, tj), qh(h, 0, NT), start=True, stop=True,
                    )
            pt_t = pts.tile([P, 1024], BF16, tag="pt", name=f"tpt{hp}")
            nc.scalar.activation(pt_t[:], st_t[:], EXP)
            # independent PE work while the template exp runs on ACT
            if hp == 0:
                emit_qk_chunk(0, 0, 512, 512)  # q 512:1024, gates cj0 scores
            elif pend[0]:
                emit_filler(*pend[0].pop(0))
            tos = []
            for hi, h in enumerate((h0, h1)):
                to = ps_fill.tile([P, 512], F32, tag="fill", name=f"to{h}")
                for tj in range(2):
                    nc.tensor.matmul(
                        to[:, :NT], v_sb[:, tj, h, :],
                        pt_t[:, hi * 512 + tj * NT: hi * 512 + (tj + 1) * NT],
                        start=(tj == 0), stop=(tj == 1),
                    )
                tos.append(to)
            normalize_pair((h0, h1), tos, 0, NT)

            # search: queries [256, 1280) attend all keys, chunk-outer.
            # Software-pipelined by one tile: scores(tk+1) and the fillers
            # run on the PE while exp(tk) runs on ACT, then AV(tk).
            def emit_scores(c0, cw, tag):
                # each head's scores go to its own PSUM bank (h1 at col 512
                # even when cw < 512): the two matmuls run concurrently and
                # concurrent PE writes into one bank are a fatal collision.
                st = ps_sc.tile([P, 1024], F32, tag="sc", name=f"st{tag}")
                for hi, h in enumerate((h0, h1)):
                    nc.tensor.matmul(
                        st[:, hi * 512:hi * 512 + cw],
                        kh(h, tag[2]), qh(h, c0, cw), start=True, stop=True,
                    )
                pt = pts.tile([P, 1024], BF16, tag="pt", name=f"pt{tag}")
                st_v = st[:, :].rearrange("p (b c) -> p b c", b=2)[:, :, 0:cw]
                pt_v = pt[:, :2 * cw].rearrange("p (b c) -> p b c", b=2)
                nc.scalar.activation(pt_v, st_v, EXP)
                return pt

            for ci, (c0, cw) in enumerate(chunks):
                pending = pend[ci]
                npend = len(pending)
                # pop slots spread over the window INCLUDING both endpoints:
                # a filler at tk=0 covers the first AV's exp wait, one at
                # tk=9 covers the pipeline drain before the next block
                slots = sorted(round(i * (TT - 1) / max(npend - 1, 1))
                               for i in range(npend))
                drained = 0
                ots = {h: ps_ot.tile([P, 512], F32, tag="ot",
                                     name=f"ot{h}_{ci}")
                       for h in (h0, h1)}
                pt = emit_scores(c0, cw, (hp, ci, 0))
                for tk in range(TT):
                    pt_next = (emit_scores(c0, cw, (hp, ci, tk + 1))
                               if tk + 1 < TT else None)
                    # drain fillers at their assigned slots (between the
                    # exp(tk) producer and its AV consumer)
                    while drained < npend and slots[drained] <= tk:
                        emit_filler(*pending[drained])
                        drained += 1
                    for hi, h in enumerate((h0, h1)):
                        nc.tensor.matmul(
                            ots[h][:, :cw], v_sb[:, tk, h, :],
                            pt[:, hi * cw:(hi + 1) * cw],
                            start=(tk == 0), stop=(tk == TT - 1),
                        )
                    pt = pt_next
                normalize_pair((h0, h1), (ots[h0], ots[h1]), c0, cw)

        # tail: only the ct=5 proj term + combine for tiles 8,9
        # (interleaved so both output-DMA rings drain in parallel)
        for half in (0, 1):
            for tt in (8, 9):
                emit_proj_final(tt, half)

    nc.compile()
    return nc


_NC = None


def _get_nc():
    global _NC
    if _NC is None:
        _NC = build_nc()
    return _NC


def _prep_in_maps(x, qkv_w, proj_w, proj_b):
    """Host-side layout: transpose, cast to bf16, and arrange DRAM images
    to match the SBUF tiles flat per partition (multi-KB DMA runs)."""
    bf16 = ml_dtypes.bfloat16
    x = np.asarray(x, dtype=np.float32)
    wt = np.asarray(qkv_w, np.float32).T          # [768, 2304]
    # q/k block region, reordered so pair-0 q/k leads
    a = wt[:, :12 * P].reshape(CT, P, 12, P).transpose(1, 2, 0, 3)
    wqk = a[:, WQK_ORDER]                          # [128, 12, 6, 128]
    # v regions
    v0 = wt[:, 12 * P:12 * P + 512].reshape(CT, P, 512).transpose(1, 0, 2)
    v1 = wt[:, 12 * P + 512:].reshape(CT, P, 256).transpose(1, 0, 2)
    wdram = np.concatenate(
        [wqk.reshape(P, -1), v0.reshape(P, -1), v1.reshape(P, -1)],
        axis=1).astype(bf16)

    pw = np.asarray(proj_w, np.float32).T.reshape(CT, P, C).transpose(1, 0, 2)
    pwdram = np.ascontiguousarray(pw).reshape(P, -1).astype(bf16)
    pb = np.ascontiguousarray(np.asarray(proj_b, np.float32)).reshape(1, C)

    maps = []
    for i in range(8):
        b = x[i].T.reshape(CT, P, NTOK).transpose(1, 0, 2)  # [128, 6, 1280]
        xdram = np.zeros((P, 2, CT, C), np.float32)
        xdram[:, 0, :, 0:512] = b[:, :, 0:512]
        xdram[:, 1, :, :] = b[:, :, 512:]
        maps.append({
            "xT": xdram.reshape(P, -1).astype(bf16),
            "wT": wdram, "pwT": pwdram, "pb": pb,
        })
    return maps


def kernel(x, qkv_w, proj_w, proj_b, **_ignored):
    from concourse.bass_utils import run_bass_kernel_spmd

    nc = _get_nc()
    res = run_bass_kernel_spmd(nc, _prep_in_maps(x, qkv_w, proj_w, proj_b),
                               list(range(8)))
    return np.stack([res.results[i]["out"] for i in range(8)]).astype(np.float32)


if __name__ == "__main__":
    rng = np.random.default_rng(0)
    ins = {
        "x": rng.standard_normal((8, NTOK, C), dtype=np.float32),
        "qkv_w": rng.standard_normal((3 * C, C), dtype=np.float32) * 0.02,
        "proj_w": rng.standard_normal((C, C), dtype=np.float32) * 0.02,
        "proj_b": np.zeros(C, dtype=np.float32),
    }
    out = kernel(**ins)
    print("out", out.shape, out.dtype)


# revision 60
# speedup vs baseline: 1.0050x; 1.0050x over previous
"""Sparse attention (template/search) Trainium2 kernel.

Model (per batch b):
  qkv = x @ qkv_w.T                  -> split to q, k, v heads (12 heads, hd=64)
  template tokens   [0, 256)  attend to template keys only
  search   tokens [256, 1280) attend to all 1280 keys
  out = softmax(q k^T / 8) v   per head, concat heads, @ proj_w.T + proj_b

Sharding: data-parallel over batch, one batch per NeuronCore (8 cores).
No collectives needed.

Layout: all transposes + fp32->bf16 casts are done on the HOST, and the
DRAM images are laid out to match the SBUF destinations byte-for-byte, so
every input DMA is a flat [128, range] copy with multi-KB contiguous runs.
Each region is split across BOTH HWDGE rings (sync + scalar share the
~358 GB/s HBM budget) and ordered by first-use deadline, so attention
starts ~6us into the kernel.  The output is written bf16 (upcast to fp32
on the host), halving output DMA traffic.

Per-core structure:
  - q,k computed feature-major into a 2-slot rotating buffer (slot =
    pair%2): qk[P, slot, {q,k}, NTOK] (q pre-scaled by 1/8).
  - v computed token-major, augmented per head as [1 | 63 zeros | v]:
    row 0 of the AV output is the softmax denominator.
  - scores computed TRANSPOSED: S.T[tk, tq] = K_h @ Q_h.T.  The two heads
    of a pair sit on PE row groups 0-63 / 64-127, so their score matmuls
    run CONCURRENTLY (hw-verified: a packed pair costs ~232ns vs 217ns for
    one matmul), filling the two halves of one PSUM tile -> ONE exp
    instruction per (pair, chunk, tk) covers both heads.
  - search is chunk-outer over query chunks so each head's AV accumulator
    is one PSUM bank; the tk loop is software-pipelined: scores(tk+1) and
    a filler run on the PE while exp(tk) runs on ACT, then AV(tk).
  - fillers (qkv / v / proj matmuls) are spread evenly over the tk slots
    so the PE never drains at chunk ends.
  - pair 5 splits its search into (512, 256, 256) query chunks and the
    tail tiles' proj is pre-accumulated (ct 0..4) as fillers, so after the
    very last normalize only 4 tiny ct=5 matmuls + combines + 0.8 MB of
    output DMA remain.
  - normalize fully off the ACT queue: DVE copy PSUM->SBUF, gpsimd
    partition_broadcast of row 0, DVE approx reciprocal, DVE multiply.
  - PE HAM warmup: zero matmuls at t=0 (no DMA dependency) open the clock
    gate while the input DMA streams.
"""

import numpy as np
import ml_dtypes

import concourse.bacc as bacc
import concourse.mybir as mybir
import concourse.tile as tile

P = 128
NTOK = 1280
C = 768
H = 12
HD = 64
NT = 256          # template tokens  [0, NT)
TT = NTOK // P    # 10 token tiles
CT = C // P       # 6 channel tiles
SCALE = HD ** -0.5

F32 = mybir.dt.float32
BF16 = mybir.dt.bfloat16
EXP = mybir.ActivationFunctionType.Exp
MULT = mybir.AluOpType.mult
ADD = mybir.AluOpType.add

# q/k weight block order in the DRAM/SBUF image: pair-0 q and k first
# (the DMA priority prefix), then q1..q5, k1..k5.  Entry i = original
# column-block (of qkv_w.T's 12 leading 128-col blocks) stored at slot i.
WQK_ORDER = [0, 6, 1, 2, 3, 4, 5, 7, 8, 9, 10, 11]
WQK_SLOT = {orig: slot for slot, orig in enumerate(WQK_ORDER)}


def build_nc():
    from contextlib import ExitStack

    nc = bacc.Bacc("TRN2", target_bir_lowering=False, debug=False, num_devices=8)
    # host-prepared flat images (see _prep_in_maps)
    xT_ext = nc.dram_tensor("xT", [P, 2 * CT * C], BF16, kind="ExternalInput")
    wT_ext = nc.dram_tensor("wT", [P, 18 * CT * P], BF16, kind="ExternalInput")
    pwT_ext = nc.dram_tensor("pwT", [P, CT * C], BF16, kind="ExternalInput")
    pb_ext = nc.dram_tensor("pb", [1, C], F32, kind="ExternalInput")
    out_ext = nc.dram_tensor("out", [NTOK, C], BF16, kind="ExternalOutput")

    with tile.TileContext(nc) as tc, ExitStack() as ctx:
        const = ctx.enter_context(tc.tile_pool(name="const", bufs=1))
        big = ctx.enter_context(tc.tile_pool(name="big", bufs=1))

        zb = const.tile([P, P], BF16)
        nc.gpsimd.memset(zb[:], 0.0)
        bias_bc = const.tile([P, C], F32)
        bias_row = const.tile([1, C], F32)

        # x, feature-major, split [tokens 0:512 | tokens 512:1280(+pad)]
        xt = big.tile([P, 2, CT, C], BF16)
        wqk = big.tile([P, 12, CT, P], BF16)   # q/k weights, WQK_ORDER blocks
        wv0 = big.tile([P, CT, 512], BF16)     # v weights cols 0:512 (h 0-7)
        wv1 = big.tile([P, CT, 256], BF16)     # v weights cols 512:768
        pwT = big.tile([P, CT, C], BF16)       # proj_w.T

        # ---- input DMA: flat 2D copies matching the host DRAM image,
        # priority-ordered and split across the two HWDGE queues ----
        xtf = xt[:, :, :, :].rearrange("p a b c -> p (a b c)")
        wqkf = wqk[:, :, :, :].rearrange("p a b c -> p (a b c)")
        wv0f = wv0[:, :, :].rearrange("p a b -> p (a b)")
        wv1f = wv1[:, :, :].rearrange("p a b -> p (a b)")
        pwf = pwT[:, :, :].rearrange("p a b -> p (a b)")
        QK0 = 2 * CT * P          # pair-0 q/k prefix elems
        XH = CT * C               # one x half
        XQ = XH // 2              # one x quarter (3 channel tiles)
        WR = (QK0 + 12 * CT * P) // 2   # midpoint of the q/k remainder
        WV = 12 * CT * P
        # both HWDGE rings share the ~358 GB/s HBM budget; split every
        # ramp-critical region across the two rings so each lands in half
        # the time, ordered by first-use deadline: pair-0 q/k + x half 0
        # (first qkv matmuls, t~6us), x half 1 (template-gap q chunk,
        # t~8us), v weights half 0 (template AV, t~10us), the rest
        nc.sync.dma_start(wqkf[:, 0:QK0], wT_ext.ap()[:, 0:QK0])
        nc.scalar.dma_start(xtf[:, 0:XQ], xT_ext.ap()[:, 0:XQ])
        nc.sync.dma_start(xtf[:, XQ:XH], xT_ext.ap()[:, XQ:XH])
        nc.scalar.dma_start(xtf[:, XH:XH + XQ], xT_ext.ap()[:, XH:XH + XQ])
        nc.sync.dma_start(xtf[:, XH + XQ:], xT_ext.ap()[:, XH + XQ:])
        WVH = WV + CT * 256
        nc.scalar.dma_start(wv0f[:, 0:CT * 256], wT_ext.ap()[:, WV:WVH])
        nc.sync.dma_start(wv0f[:, CT * 256:], wT_ext.ap()[:, WVH:WV + CT * 512])
        nc.scalar.dma_start(wqkf[:, QK0:WR], wT_ext.ap()[:, QK0:WR])
        nc.sync.dma_start(wqkf[:, WR:], wT_ext.ap()[:, WR:12 * CT * P])
        nc.scalar.dma_start(wv1f[:], wT_ext.ap()[:, WV + CT * 512:])
        nc.sync.dma_start(pwf[:], pwT_ext.ap())
        nc.scalar.dma_start(bias_row[:], pb_ext.ap())
        nc.gpsimd.partition_broadcast(bias_bc[:], bias_row[0:1, :])

        def xchunk(ct, c0, cw):
            """x.T [ct*128:(ct+1)*128, c0:c0+cw] from the split layout
            (chunks never straddle the 512 boundary)."""
            if c0 + cw <= 512:
                return xt[:, 0, ct, c0:c0 + cw]
            return xt[:, 1, ct, c0 - 512:c0 - 512 + cw]

        big2 = ctx.enter_context(tc.tile_pool(name="big2", bufs=1))
        # q (scaled) and k, feature-major, 2-slot rotation keyed by pair%2
        qk = big2.tile([P, 2, 2, NTOK], BF16)
        v_sb = big2.tile([P, TT, H, P], BF16)  # [1 | 63 zeros | v] per head
        ot_all = big2.tile([P, CT, NTOK], BF16)     # attention out, feature-major
        out_sb = big2.tile([P, TT, C], BF16)
        # proj partial sums (ct 0..4 terms + bias) for the tail tiles 6..9
        proj_part = big2.tile([P, 4, C], F32)

        # v_aug layout per head: col 0 = ones (softmax denominator row),
        # cols 1:64 = zeros (padding so O lands at partitions 64:128)
        nc.gpsimd.memset(v_sb[:, :, :, 0:64], 0.0)
        nc.gpsimd.memset(v_sb[:, :, :, 0:1], 1.0)

        # ---- HAM warmup: zero matmuls with no DMA dependency keep the PE
        # busy from t=0 so its clock gate opens (1.2 -> 2.4 GHz) before the
        # first real matmul.  Result (zeros) lands in v_sb zero padding to
        # keep the chain live.
        with tc.tile_pool(name="warm", bufs=2, space="PSUM") as ps_warm:
            wps = [ps_warm.tile([P, P], F32, name=f"wps{j}") for j in range(2)]
            for i in range(56):
                nc.tensor.matmul(wps[i % 2][:], zb[:], zb[:],
                                 start=True, stop=True)
            nc.vector.tensor_copy(v_sb[:, 0, 0, 1:64], wps[1][:, 0:63])

        ps_fill = ctx.enter_context(tc.tile_pool(name="ps_fill", bufs=2, space="PSUM"))

        # ---- qkv projection (emitted interleaved with attention below) ----
        def emit_qk_chunk(hp, which, c0, cw):
            """qk[slot, which] = (q|k) row block of head pair hp,
            feature-major, for token chunk [c0, c0+cw)."""
            slot = WQK_SLOT[hp + 6 * which]
            ps = ps_fill.tile([P, 512], F32, tag="fill", name=f"qkp{hp}_{which}_{c0}")
            for ct in range(CT):
                nc.tensor.matmul(
                    ps[:, :cw],
                    wqk[:, slot, ct, :],
                    xchunk(ct, c0, cw),
                    start=(ct == 0), stop=(ct == CT - 1),
                )
            if which == 0:  # q: fold in softmax scale
                nc.vector.tensor_scalar_mul(
                    qk[:, hp % 2, 0, c0:c0 + cw], ps[:, :cw], SCALE
                )
            else:
                nc.vector.tensor_copy(qk[:, hp % 2, 1, c0:c0 + cw], ps[:, :cw])

        def qk_pair_chunks(p):
            return [(p, w, c0, cw)
                    for c0, cw in ((0, 512), (512, 512), (1024, 256))
                    for w in (0, 1)]

        # v token-major: v[tok, f] = x @ qkv_w.T cols [1536, 2304)
        def emit_v_chunk(tt, half):
            cw, h0, nh = ((512, 0, 8), (256, 8, 4))[half]
            wv = (wv0, wv1)[half]
            ps = ps_fill.tile([P, 512], F32, tag="fill", name=f"vp{tt}_{half}")
            for ct in range(CT):
                nc.tensor.matmul(
                    ps[:, :cw],
                    xchunk(ct, tt * P, P),
                    wv[:, ct, :],
                    start=(ct == 0), stop=(ct == CT - 1),
                )
            nc.vector.tensor_copy(
                v_sb[:, tt, h0:h0 + nh, 64:128],
                ps[:, :cw].rearrange("p (h e) -> p h e", e=HD),
            )

        # ---- output projection ----
        def emit_proj_chunk(tt, half):
            c0, cw = ((0, 512), (512, 256))[half]
            ps = ps_fill.tile([P, 512], F32, tag="fill", name=f"prj{tt}_{c0}")
            for ct in range(CT):
                nc.tensor.matmul(
                    ps[:, :cw],
                    ot_all[:, ct, tt * P:(tt + 1) * P],
                    pwT[:, ct, c0:c0 + cw],
                    start=(ct == 0), stop=(ct == CT - 1),
                )
            nc.vector.tensor_tensor(
                out_sb[:, tt, c0:c0 + cw], ps[:, :cw],
                bias_bc[:, c0:c0 + cw], ADD,
            )
            nc.sync.dma_start(out_ext.ap()[tt * P:(tt + 1) * P, c0:c0 + cw],
                              out_sb[:, tt, c0:c0 + cw])

        # proj split for the tail tiles (6..9): ct 0..4 terms accumulate as
        # pair-5 fillers (pairs 0-4 are fully normalized by then); the ct=5
        # term + combine runs after that tile's chunk normalize.
        def emit_proj_partial(tt, half):
            c0, cw = ((0, 512), (512, 256))[half]
            ps = ps_fill.tile([P, 512], F32, tag="fill", name=f"prp{tt}_{c0}")
            for ct in range(CT - 1):
                nc.tensor.matmul(
                    ps[:, :cw],
                    ot_all[:, ct, tt * P:(tt + 1) * P],
                    pwT[:, ct, c0:c0 + cw],
                    start=(ct == 0), stop=(ct == CT - 2),
                )
            nc.vector.tensor_tensor(
                proj_part[:, tt - 6, c0:c0 + cw], ps[:, :cw],
                bias_bc[:, c0:c0 + cw], ADD,
            )

        def emit_proj_final(tt, half):
            c0, cw = ((0, 512), (512, 256))[half]
            ps = ps_fill.tile([P, 512], F32, tag="fill", name=f"prf{tt}_{c0}")
            nc.tensor.matmul(
                ps[:, :cw],
                ot_all[:, CT - 1, tt * P:(tt + 1) * P],
                pwT[:, CT - 1, c0:c0 + cw],
                start=True, stop=True,
            )
            nc.vector.tensor_tensor(
                out_sb[:, tt, c0:c0 + cw], ps[:, :cw],
                proj_part[:, tt - 6, c0:c0 + cw], ADD,
            )
            # tail tiles alternate between the two HWDGE rings to double
            # the end-of-kernel output DMA bandwidth
            eng = nc.scalar if tt % 2 else nc.sync
            eng.dma_start(out_ext.ap()[tt * P:(tt + 1) * P, c0:c0 + cw],
                          out_sb[:, tt, c0:c0 + cw])

        def emit_filler(kind, arg):
            if kind == "qk":
                emit_qk_chunk(*arg)
            elif kind == "v":
                emit_v_chunk(*arg)
            elif kind == "projp":
                emit_proj_partial(*arg)
            elif kind == "projf":
                emit_proj_final(*arg)
            else:
                emit_proj_chunk(*arg)

        # pair-0 q/k c0 chunks + template v tiles up front (these gate the
        # first template block and only need the priority DMAs); the rest
        # of pair-0 q/k streams inside pair 0 itself
        emit_qk_chunk(0, 0, 0, 512)
        emit_qk_chunk(0, 1, 0, 512)
        for tt in (0, 1):
            emit_v_chunk(tt, 0)

        # ---- attention ----
        ps_sc = ctx.enter_context(tc.tile_pool(name="ps_sc", bufs=2, space="PSUM"))
        ps_ot = ctx.enter_context(tc.tile_pool(name="ps_ot", bufs=2, space="PSUM"))
        pts = ctx.enter_context(tc.tile_pool(name="pts", bufs=4))
        dn = ctx.enter_context(tc.tile_pool(name="dn", bufs=2))
        rbp = ctx.enter_context(tc.tile_pool(name="rbp", bufs=2))

        def qh(h, c0, cw):
            b = (h % 2) * 64
            return qk[b:b + 64, (h // 2) % 2, 0, c0:c0 + cw]

        def kh(h, tk):
            b = (h % 2) * 64
            return qk[b:b + 64, (h // 2) % 2, 1, tk * P:(tk + 1) * P]

        def normalize_pair(hs, ot_pss, c0, cw):
            """Per head: ot_ps [128, cw] psum (row 0 = denominators, rows
            64:128 = O.T for tq cols [c0, c0+cw)).  Normalize and write to
            ot_all, fully off the ACT queue.  Both heads' PSUM->SBUF copies
            are issued first so the second head's chain is not blocked
            behind the first head's broadcast wait on the DVE FIFO."""
            dens, rbs = [], []
            for h, ot_ps in zip(hs, ot_pss):
                den = dn.tile([P, 512], F32, tag="dn")
                nc.vector.tensor_copy(den[:, :cw], ot_ps[:, :cw])
                dens.append(den)
            for den in dens:
                rb = rbp.tile([P, 512], F32, tag="rb")
                nc.gpsimd.partition_broadcast(rb[:, :cw], den[0:1, :cw])
                rbs.append(rb)
            for h, den, rb in zip(hs, dens, rbs):
                b = (h % 2) * 64
                nc.vector.reciprocal_approx_fast(rb[:, :cw], rb[:, :cw])
                nc.vector.tensor_tensor(
                    ot_all[b:b + 64, h // 2, c0:c0 + cw],
                    den[64:128, :cw], rb[64:128, :cw], MULT,
                )

        for hp in range(6):
            h0, h1 = 2 * hp, 2 * hp + 1
            # query chunks of this pair's search and their filler lists
            if hp < 5:
                chunks = [(NT, 512), (NT + 512, 512)]
            else:
                chunks = [(NT, 512), (NT + 512, 256), (NT + 768, 256)]
            if hp == 0:
                # pair-0's own remaining q/k interleaves with its search:
                # k(512) needed at tk4, k(1024) at tk8, q(1024) before cj1
                pend = [[("qk", (0, 1, 512, 512)),
                         ("v", (2, 0)), ("v", (3, 0)), ("v", (4, 0)),
                         ("qk", (0, 1, 1024, 256))]
                        + [("v", (tt, 0)) for tt in range(5, TT)]
                        + [("qk", (0, 0, 1024, 256))],
                        [("qk", a) for a in qk_pair_chunks(1)]]
            elif hp < 4:
                nxt = [("qk", a) for a in qk_pair_chunks(hp + 1)]
                vh = {1: [0, 1, 2, 3], 2: [4, 5, 6], 3: [7, 8, 9]}[hp]
                va = [("v", (tt, 1)) for tt in vh]
                pend = [nxt[:3] + va[:2], nxt[3:] + va[2:]]
            elif hp == 4:
                nxt = [("qk", a) for a in qk_pair_chunks(5)]
                pend = [nxt[:3], nxt[3:]]
            else:
                # chunk 0 (q 256:768), chunk A (768:1024 -> tiles 6,7),
                # chunk B (1024:1280 -> tiles 8,9).  proj gates: tiles 0,1
                # by the template normalize just below; tiles 2-5 by chunk-0
                # normalize; prp by pairs 0-4 (already done); prf(6,7) by
                # chunk-A normalize.
                pend = [[("projp", (6, 0)), ("projp", (6, 1)),
                         ("proj", (0, 0)), ("proj", (0, 1)),
                         ("proj", (1, 0)), ("proj", (1, 1)),
                         ("projp", (7, 0)), ("projp", (7, 1)),
                         ("projp", (8, 0)), ("projp", (8, 1))],
                        [("projp", (9, 0)), ("projp", (9, 1)),
                         ("proj", (2, 0)), ("proj", (2, 1)),
                         ("proj", (3, 0)), ("proj", (3, 1)),
                         ("proj", (4, 0)), ("proj", (4, 1))],
                        [("proj", (5, 0)), ("proj", (5, 1)),
                         ("projf", (6, 0)), ("projf", (6, 1)),
                         ("projf", (7, 0)), ("projf", (7, 1))]]

            # template block, both heads fused: queries [0,256) x keys [0,256)
            st_t = ps_sc.tile([P, 1024], F32, tag="sc", name=f"tst{hp}")
            for tj in range(2):
                for hi, h in enumerate((h0, h1)):
                    nc.tensor.matmul(
                        st_t[:, hi * 512 + tj * NT: hi * 512 + (tj + 1) * NT],
                        kh(h, tj), qh(h, 0, NT), start=True, stop=True,
                    )
            pt_t = pts.tile([P, 1024], BF16, tag="pt", name=f"tpt{hp}")
            nc.scalar.activation(pt_t[:], st_t[:], EXP)
            # independent PE work while the template exp runs on ACT
            if hp == 0:
                emit_qk_chunk(0, 0, 512, 512)  # q 512:1024, gates cj0 scores
            elif pend[0]:
                emit_filler(*pend[0].pop(0))
            tos = []
            for hi, h in enumerate((h0, h1)):
                to = ps_fill.tile([P, 512], F32, tag="fill", name=f"to{h}")
                for tj in range(2):
                    nc.tensor.matmul(
                        to[:, :NT], v_sb[:, tj, h, :],
                        pt_t[:, hi * 512 + tj * NT: hi * 512 + (tj + 1) * NT],
                        start=(tj == 0), stop=(tj == 1),
                    )
                tos.append(to)
            normalize_pair((h0, h1), tos, 0, NT)

            # search: queries [256, 1280) attend all keys, chunk-outer.
            # Software-pipelined by one tile: scores(tk+1) and the fillers
            # run on the PE while exp(tk) runs on ACT, then AV(tk).
            def emit_scores(c0, cw, tag):
                # each head's scores go to its own PSUM bank (h1 at col 512
                # even when cw < 512): the two matmuls run concurrently and
                # concurrent PE writes into one bank are a fatal collision.
                st = ps_sc.tile([P, 1024], F32, tag="sc", name=f"st{tag}")
                for hi, h in enumerate((h0, h1)):
                    nc.tensor.matmul(
                        st[:, hi * 512:hi * 512 + cw],
                        kh(h, tag[2]), qh(h, c0, cw), start=True, stop=True,
                    )
                pt = pts.tile([P, 1024], BF16, tag="pt", name=f"pt{tag}")
                st_v = st[:, :].rearrange("p (b c) -> p b c", b=2)[:, :, 0:cw]
                pt_v = pt[:, :2 * cw].rearrange("p (b c) -> p b c", b=2)
                nc.scalar.activation(pt_v, st_v, EXP)
                return pt

            for ci, (c0, cw) in enumerate(chunks):
                pending = pend[ci]
                npend = len(pending)
                # pop slots spread over the window INCLUDING both endpoints:
                # a filler at tk=0 covers the first AV's exp wait, one at
                # tk=9 covers the pipeline drain before the next block
                slots = sorted(round(i * (TT - 1) / max(npend - 1, 1))
                               for i in range(npend))
                drained = 0
                ots = {h: ps_ot.tile([P, 512], F32, tag="ot",
                                     name=f"ot{h}_{ci}")
                       for h in (h0, h1)}
                pt = emit_scores(c0, cw, (hp, ci, 0))
                for tk in range(TT):
                    pt_next = (emit_scores(c0, cw, (hp, ci, tk + 1))
                               if tk + 1 < TT else None)
                    # drain fillers at their assigned slots (between the
                    # exp(tk) producer and its AV consumer)
                    while drained < npend and slots[drained] <= tk:
                        emit_filler(*pending[drained])
                        drained += 1
                    for hi, h in enumerate((h0, h1)):
                        nc.tensor.matmul(
                            ots[h][:, :cw], v_sb[:, tk, h, :],
                            pt[:, hi * cw:(hi + 1) * cw],
                            start=(tk == 0), stop=(tk == TT - 1),
                        )
                    pt = pt_next
                normalize_pair((h0, h1), (ots[h0], ots[h1]), c0, cw)

        # tail: only the ct=5 proj term + combine for tiles 8,9
        # (interleaved so both output-DMA rings drain in parallel)
        for half in (0, 1):
            for tt in (8, 9):
                emit_proj_final(tt, half)

    nc.compile()
    return nc


_NC = None


def _get_nc():
    global _NC
    if _NC is None:
        _NC = build_nc()
    return _NC


def _prep_in_maps(x, qkv_w, proj_w, proj_b):
    """Host-side layout: transpose, cast to bf16, and arrange DRAM images
    to match the SBUF tiles flat per partition (multi-KB DMA runs)."""
    bf16 = ml_dtypes.bfloat16
    x = np.asarray(x, dtype=np.float32)
    wt = np.asarray(qkv_w, np.float32).T          # [768, 2304]
    # q/k block region, reordered so pair-0 q/k leads
    a = wt[:, :12 * P].reshape(CT, P, 12, P).transpose(1, 2, 0, 3)
    wqk = a[:, WQK_ORDER]                          # [128, 12, 6, 128]
    # v regions
    v0 = wt[:, 12 * P:12 * P + 512].reshape(CT, P, 512).transpose(1, 0, 2)
    v1 = wt[:, 12 * P + 512:].reshape(CT, P, 256).transpose(1, 0, 2)
    wdram = np.concatenate(
        [wqk.reshape(P, -1), v0.reshape(P, -1), v1.reshape(P, -1)],
        axis=1).astype(bf16)

    pw = np.asarray(proj_w, np.float32).T.reshape(CT, P, C).transpose(1, 0, 2)
    pwdram = np.ascontiguousarray(pw).reshape(P, -1).astype(bf16)
    pb = np.ascontiguousarray(np.asarray(proj_b, np.float32)).reshape(1, C)

    maps = []
    for i in range(8):
        b = x[i].T.reshape(CT, P, NTOK).transpose(1, 0, 2)  # [128, 6, 1280]
        xdram = np.zeros((P, 2, CT, C), np.float32)
        xdram[:, 0, :, 0:512] = b[:, :, 0:512]
        xdram[:, 1, :, :] = b[:, :, 512:]
        maps.append({
            "xT": xdram.reshape(P, -1).astype(bf16),
            "wT": wdram, "pwT": pwdram, "pb": pb,
        })
    return maps


def kernel(x, qkv_w, proj_w, proj_b, **_ignored):
    from concourse.bass_utils import run_bass_kernel_spmd

    nc = _get_nc()
    res = run_bass_kernel_spmd(nc, _prep_in_maps(x, qkv_w, proj_w, proj_b),
                               list(range(8)))
    return np.stack([res.results[i]["out"] for i in range(8)]).astype(np.float32)


if __name__ == "__main__":
    rng = np.random.default_rng(0)
    ins = {
        "x": rng.standard_normal((8, NTOK, C), dtype=np.float32),
        "qkv_w": rng.standard_normal((3 * C, C), dtype=np.float32) * 0.02,
        "proj_w": rng.standard_normal((C, C), dtype=np.float32) * 0.02,
        "proj_b": np.zeros(C, dtype=np.float32),
    }
    out = kernel(**ins)
    print("out", out.shape, out.dtype)
